# revision 1
# baseline (speedup 1.0000x reference)
"""DiT block kernel for 8 Trainium2 NeuronCores.

Sharding: core = (b, half) with b = core//2 in [0,4), half = core%2.
Each core computes output rows [half*1024:(half+1)*1024) of batch b.
K/V are computed for the full 2048 rows of the batch on both cores of a
pair (duplicated compute, no collectives needed).

Host-side prep folds the adaLN modulation into per-core weight/vector
inputs:
  mod = silu(c) @ W_ada + b_ada -> scale_msa, gate_msa, scale_mlp, gate_mlp
  g1' = g1*(1+scale_msa[b]),  b1' = b1*(1+scale_msa[b])
  g2' = g2*(1+scale_mlp[b]),  b2' = b2*(1+scale_mlp[b])
  Wo' = Wo * gate_msa[b][None,:]
  W2' = W_mlp2 * gate_mlp[b][None,:],  b2m' = b_mlp2*gate_mlp[b]
fp8 weights (Wq/Wk/Wv/W1/W2) are pre-scaled by 8 on the host so their
values sit in the f8e4m3 normal range; the inverse scale is folded into
free scale slots downstream (exp scale for attention, ACT scale for
gelu, the output tensor_scalar for W2).

Device pipeline (single layer, one shot):
  - LN1 over 16 row tiles; rstd = exp(-0.5*ln(var+eps)) so the ACT
    engine only ever needs the natural_log_exp table set until gelu.
  - QKV with fp8 DoubleRow, ordered so attention's first consumers
    (kT chunk 0, qT chunk 0, v tiles) are produced first; PSUM->SBUF
    copies run on DVE and overlap into the attention window.
  - Attention with transposed scores, exp on ACT (scale folds 1/sqrt(dk)
    and the 8*8 weight prescale), PV in fp8 DoubleRow with a ones column
    (value 8.0) appended to v giving the softmax denominator.
  - Softmax normalization: DVE reciprocal of the denominator row, an
    unnormalized PV copy to SBUF (frees the PSUM bank immediately), a
    partition-broadcast of the reciprocal via a DRAM roundtrip on the SP
    queue (decoupled from the PE/exp pipeline), multiply on DVE. The
    last head-pair of qb1 gates the tail and uses a K=1 PE rank-1
    matmul instead of the roundtrip.
  - Wo (bf16) + residual + LN2 per query block, overlapped with the
    other query block's attention. MLP in fp8 DoubleRow, gelu on ACT.
"""

import numpy as np
import ml_dtypes

P = 128
D = 512
T = 2048
TL = 1024  # rows per core
H = 8
DK = 64
HID = 2048
EPS = 1e-5
NCORES = 8
WS = 8.0  # host-side fp8 weight prescale

BF16 = ml_dtypes.bfloat16
F8E4 = ml_dtypes.float8_e4m3

_cache = {}


def _split_excess_waits(nc):
    """This walrus build accepts at most 1 sync wait per instruction (2 for
    EVENT_SEMAPHORE), but Tile can attach more. Move excess waits onto
    ENGINE_NOP carriers inserted just before the instruction on the same
    engine (sequencer program order preserves the wait semantics)."""
    from concourse import mybir

    Op = nc.isa.Opcode

    def is_evsem(inst):
        return (
            isinstance(inst, mybir.InstISA)
            and inst.isa_opcode == Op.NEURON_ISA_TPB_OPCODE_EVENT_SEMAPHORE.value
        ) or "EventSem" in type(inst).__name__

    for f in nc.m.functions:
        for bb in f.blocks:
            out = []
            changed = False
            for inst in bb.instructions:
                si = inst.sync_info
                keep = 2 if is_evsem(inst) else 1
                if si is not None and len(si.on_wait) > keep:
                    excess = list(si.on_wait[:-keep])
                    del si.on_wait[:-keep]
                    for w in excess:
                        n = nc.engines[inst.engine]._isa(
                            Op.NEURON_ISA_TPB_OPCODE_NOP, {}
                        )
                        n.sync_info = mybir.SyncInfo(on_wait=[w], on_update=[])
                        out.append(n)
                    changed = True
                out.append(inst)
            if changed:
                bb.instructions[:] = out


def _bcast_ap(bass, src_ap, parts):
    """AP with the partition dim replaced by a step-0 broadcast."""
    return bass.AP(
        tensor=src_ap.tensor,
        offset=src_ap.offset,
        ap=[[0, parts], *[list(d) for d in src_ap.ap[1:]]],
    )


def _bcast_vec_ap(bass, dram_ap, parts):
    """[n] DRAM AP -> [parts, n] partition-broadcast AP."""
    return bass.AP(
        tensor=dram_ap.tensor,
        offset=dram_ap.offset,
        ap=[[0, parts], *dram_ap.ap],
    )


def _build_program(has_b1, has_b2, has_bm1, has_bm2):
    import concourse.bass as bass
    import concourse.tile as tile
    from concourse import mybir
    from concourse.masks import make_identity
    from contextlib import ExitStack

    f32 = mybir.dt.float32
    bf16 = mybir.dt.bfloat16
    f8 = mybir.dt.float8e4
    AF = mybir.ActivationFunctionType
    OP = mybir.AluOpType

    nc = bass.Bass("TRN2", target_bir_lowering=False, debug=False)

    # ---- I/O ----
    xm = nc.dram_tensor("xm", [TL, D], f32, kind="ExternalInput").ap()
    xo = nc.dram_tensor("xo", [TL, D], bf16, kind="ExternalInput").ap()
    wq_d = nc.dram_tensor("wq", [D, D], f8, kind="ExternalInput").ap()
    wk_d = nc.dram_tensor("wk", [D, D], f8, kind="ExternalInput").ap()
    wv_d = nc.dram_tensor("wv", [D, D], f8, kind="ExternalInput").ap()
    wo_d = nc.dram_tensor("wo", [D, D], bf16, kind="ExternalInput").ap()
    w1_d = nc.dram_tensor("w1", [D, HID], f8, kind="ExternalInput").ap()
    w2_d = nc.dram_tensor("w2", [HID, D], f8, kind="ExternalInput").ap()
    g1_d = nc.dram_tensor("g1", [D], f32, kind="ExternalInput").ap()
    g2_d = nc.dram_tensor("g2", [D], f32, kind="ExternalInput").ap()
    b1_d = nc.dram_tensor("b1", [D], f32, kind="ExternalInput").ap() if has_b1 else None
    b2_d = nc.dram_tensor("b2", [D], f32, kind="ExternalInput").ap() if has_b2 else None
    bm1_d = (
        nc.dram_tensor("bm1", [HID], f32, kind="ExternalInput").ap() if has_bm1 else None
    )
    bm2_d = (
        nc.dram_tensor("bm2", [D], f32, kind="ExternalInput").ap() if has_bm2 else None
    )
    y = nc.dram_tensor("y", [TL, D], f32, kind="ExternalOutput").ap()

    xm_t = xm.rearrange("(n p) d -> p n d", p=P)  # [128, 8, 512]
    xo_t = xo.rearrange("(n p) d -> p n d", p=P)
    y_t = y.rearrange("(n p) d -> p n d", p=P)
    wq_r = wq_d.rearrange("(o p) n -> p o n", p=P)  # [128, 4, 512]
    wk_r = wk_d.rearrange("(o p) n -> p o n", p=P)
    wv_r = wv_d.rearrange("(o p) n -> p o n", p=P)
    wo_r = wo_d.rearrange("(o p) n -> p o n", p=P)
    w1_r = w1_d.rearrange("(o p) n -> p o n", p=P)  # [128, 4, 2048]
    w2_r = w2_d.rearrange("(o p) n -> p o n", p=P)  # [128, 16, 512]

    NT = T // P  # 16 row tiles total
    NTL = TL // P  # 8 row tiles (mine)
    KS = D // P  # 4 contraction subtiles for D
    HC = HID // P  # 16 hidden chunks
    SC_EXP = 1.0 / (np.sqrt(DK) * WS * WS)  # exp scale: 1/sqrt(dk) / (8*8)

    with tile.TileContext(nc) as tc, ExitStack() as ctx:
        singles = ctx.enter_context(tc.tile_pool(name="singles", bufs=1))
        scratch = ctx.enter_context(tc.tile_pool(name="scratch", bufs=6))
        xnbp = ctx.enter_context(tc.tile_pool(name="xnb", bufs=6))
        stats = ctx.enter_context(tc.tile_pool(name="stats", bufs=12))
        expSp = ctx.enter_context(tc.tile_pool(name="expS", bufs=4))
        rbp = ctx.enter_context(tc.tile_pool(name="rb", bufs=6))
        outp = ctx.enter_context(tc.tile_pool(name="out", bufs=4))
        dramp = ctx.enter_context(tc.tile_pool(name="dram", bufs=4, space="DRAM"))
        # PSUM pool shared by QKV matmuls, Wo matmuls and LN2 transposes
        mmp = ctx.enter_context(tc.tile_pool(name="mmp", bufs=2, space="PSUM"))

        # ---- resident tiles ----
        ident = singles.tile([P, P], bf16)
        make_identity(nc, ident)
        eps_t = singles.tile([P, 1], f32)
        nc.vector.memset(eps_t, EPS)
        ones64 = singles.tile([1, DK], f32)
        nc.vector.memset(ones64, 1.0)

        g1B = singles.tile([P, D], f32)
        nc.gpsimd.dma_start(out=g1B, in_=_bcast_vec_ap(bass, g1_d, P))
        g2B = singles.tile([P, D], f32)
        nc.gpsimd.dma_start(out=g2B, in_=_bcast_vec_ap(bass, g2_d, P))
        b1B = b2B = bm2B = None
        if has_b1:
            b1B = singles.tile([P, D], f32)
            nc.gpsimd.dma_start(out=b1B, in_=_bcast_vec_ap(bass, b1_d, P))
        if has_b2:
            b2B = singles.tile([P, D], f32)
            nc.gpsimd.dma_start(out=b2B, in_=_bcast_vec_ap(bass, b2_d, P))
        if has_bm2:
            bm2B = singles.tile([P, D], f32)
            nc.gpsimd.dma_start(out=bm2B, in_=_bcast_vec_ap(bass, bm2_d, P))
        bm1_sb = None
        if has_bm1:
            bm1_sb = singles.tile([P, HC], f32)
            nc.sync.dma_start(out=bm1_sb, in_=bm1_d.rearrange("(o p) -> p o", p=P))

        x_res = singles.tile([P, NTL, D], f32)  # my rows; residual accumulator
        xo_sb = singles.tile([P, NTL, D], bf16)  # other-half rows (for K/V)
        xn1T = singles.tile([P, KS, T], f8)
        qT = singles.tile([P, KS, TL], bf16)
        kT = singles.tile([P, KS, T], bf16)
        VPAD = 80  # 2-dim byte-step must be a multiple of 16 for DoubleRow
        v_sb = singles.tile([P, H, NT, VPAD], f8)  # 8.0-ones column at [.., 64]
        nc.gpsimd.memset(v_sb[:, :, :, DK : DK + 1], WS)
        oT = singles.tile([P, KS, TL], bf16)
        xn2T = singles.tile([P, KS, TL], f8)
        hT = singles.tile([P, HC, TL], f8)

        # weights
        wq_sb = singles.tile([P, KS, D], f8)
        wk_sb = singles.tile([P, KS, D], f8)
        wv_sb = singles.tile([P, KS, D], f8)
        wo_sb = singles.tile([P, KS, D], bf16)
        w1_sb = singles.tile([P, KS, HID], f8)
        w2_sb = singles.tile([P, HC, D], f8)
        # x loads spread across four DMA queues so they land in parallel
        # (LN1 consumes them immediately); attention weights follow on the
        # Pool queue; the late-needed w1/w2 queue behind x on SP.
        nc.sync.dma_start(out=x_res[:, 0:4, :], in_=xm_t[:, 0:4, :])
        nc.scalar.dma_start(out=x_res[:, 4:8, :], in_=xm_t[:, 4:8, :])
        nc.scalar.dma_start(out=xo_sb[:, 0:4, :], in_=xo_t[:, 0:4, :])
        nc.sync.dma_start(out=xo_sb[:, 4:8, :], in_=xo_t[:, 4:8, :])
        nc.gpsimd.dma_start(out=wk_sb, in_=wk_r)
        nc.gpsimd.dma_start(out=wq_sb, in_=wq_r)
        nc.gpsimd.dma_start(out=wv_sb, in_=wv_r)
        nc.gpsimd.dma_start(out=wo_sb, in_=wo_r)
        nc.sync.dma_start(out=w1_sb, in_=w1_r)
        nc.sync.dma_start(out=w2_sb, in_=w2_r)

        # ---------- LN helpers: vector chain, then transpose+copy ----------
        def _ln_vec(xt_ap, gB, bB, i):
            st = stats.tile([P, 6], f32, tag="st")
            nc.vector.bn_stats(out=st, in_=xt_ap)
            mv = stats.tile([P, 2], f32, tag="mv")
            nc.vector.bn_aggr(out=mv, in_=st)
            # rstd = exp(-0.5 * ln(var + eps)) — stays in the ln/exp table set
            lnv = stats.tile([P, 1], f32, tag="lnv")
            nc.scalar.activation(lnv, mv[:, 1:2], AF.Ln, bias=eps_t)
            rstd = stats.tile([P, 1], f32, tag="rstd")
            nc.scalar.activation(rstd, lnv, AF.Exp, scale=-0.5)
            xc = scratch.tile([P, D], f32, tag="xc")
            # per-partition scalar operands (TensorScalarPtr) only exist on
            # DVE; the plain tensor-tensor multiply goes to GpSimd
            nc.vector.tensor_scalar(
                out=xc,
                in0=xt_ap,
                scalar1=mv[:, 0:1],
                scalar2=rstd,
                op0=OP.subtract,
                op1=OP.mult,
            )
            xnb = xnbp.tile([P, D], bf16, tag="xnb")
            nc.gpsimd.tensor_mul(out=xnb, in0=xc, in1=gB)
            if bB is not None:
                nc.vector.tensor_add(out=xnb, in0=xnb, in1=bB)
            return xnb

        def _ln_tp(xnb, xnT, i, xps, act_copy, tp_tag="tp"):
            tp = xps.tile([P, KS, P], bf16, tag=tp_tag)
            for c in range(KS):
                nc.tensor.transpose(tp[:, c, :], xnb[:, c * P : (c + 1) * P], ident)
            dst = xnT[:, :, i * P : (i + 1) * P]
            if act_copy:
                nc.scalar.copy(out=dst, in_=tp)
            else:
                nc.vector.tensor_copy(out=dst, in_=tp)

        def _ln_tile(xt_ap, gB, bB, xnT, i, xps, act_copy, tp_tag="tp"):
            xnb = _ln_vec(xt_ap, gB, bB, i)
            _ln_tp(xnb, xnT, i, xps, act_copy, tp_tag)

        # ========== Phase 1+2: LN1 interleaved with QKV production ==========
        def _kT_blk(c, nb, act=False):
            ps = mmp.tile([P, 512], f32, tag="ps")
            for ks in range(0, KS, 2):
                nc.tensor.matmul(
                    ps,
                    lhsT=wk_sb[:, ks : ks + 2, c * P : (c + 1) * P],
                    rhs=xn1T[:, ks : ks + 2, nb * 512 : (nb + 1) * 512],
                    start=(ks == 0),
                    stop=(ks == KS - 2),
                    perf_mode=mybir.MatmulPerfMode.DoubleRow,
                )
            dst = kT[:, c, nb * 512 : (nb + 1) * 512]
            if act:
                nc.scalar.copy(out=dst, in_=ps)
            else:
                nc.vector.tensor_copy(out=dst, in_=ps)

        def _qT_blk(c, nb, act=False):
            ps = mmp.tile([P, 512], f32, tag="ps")
            for ks in range(0, KS, 2):
                nc.tensor.matmul(
                    ps,
                    lhsT=wq_sb[:, ks : ks + 2, c * P : (c + 1) * P],
                    rhs=xn1T[:, ks : ks + 2, nb * 512 : (nb + 1) * 512],
                    start=(ks == 0),
                    stop=(ks == KS - 2),
                    perf_mode=mybir.MatmulPerfMode.DoubleRow,
                )
            dst = qT[:, c, nb * 512 : (nb + 1) * 512]
            if act:
                nc.scalar.copy(out=dst, in_=ps)
            else:
                nc.vector.tensor_copy(out=dst, in_=ps)

        def _v_tile(t):
            ps = mmp.tile([P, 512], f32, tag="ps")
            for ks in range(0, KS, 2):
                nc.tensor.matmul(
                    ps,
                    lhsT=xn1T[:, ks : ks + 2, t * P : (t + 1) * P],
                    rhs=wv_sb[:, ks : ks + 2, :],
                    start=(ks == 0),
                    stop=(ks == KS - 2),
                    perf_mode=mybir.MatmulPerfMode.DoubleRow,
                )
            nc.vector.tensor_copy(
                out=v_sb[:, :, t, 0:DK],
                in_=ps.rearrange("p (h d) -> p h d", d=DK),
            )

        with tc.tile_pool(name="xps", bufs=2, space="PSUM") as xps:
            for i in range(NT):
                if i < NTL:
                    xt_ap = x_res[:, i, :]
                else:
                    xt_ap = xo_sb[:, i - NTL, :]
                _ln_tile(xt_ap, g1B, b1B, xn1T, i, xps, act_copy=True)
                if i % 4 == 3:
                    # head-pair 0's kT/qT for the finished row block, with
                    # the PSUM->SBUF copy on ACT so DVE stays on LN1
                    nb = i // 4
                    _kT_blk(0, nb, act=True)
                    if nb < 2:
                        _qT_blk(0, nb, act=True)
            # v tiles are consumed progressively by PV; DVE copies them
            # while attention runs
            for t in range(NT):
                _v_tile(t)

        # ========== Phase 3..6: attention / Wo / LN2 / MLP ==========
        with (
            tc.tile_pool(name="sc", bufs=2, space="PSUM") as scp,
            tc.tile_pool(name="pvps", bufs=2, space="PSUM") as pvp,
        ):
            NTK = T // P  # 16 key subtiles

            def _attn_qb(qb, produce_qkv):
                qsl = slice(qb * 512, (qb + 1) * 512)
                for hp in range(H // 2):
                    # later chunks of kT/qT: produced one head-pair ahead,
                    # one block per g iteration so PE interleaves them with
                    # this head-pair's score matmuls
                    produce = []
                    if produce_qkv and hp < H // 2 - 1:
                        produce = [
                            lambda nb=nb: _kT_blk(hp + 1, nb) for nb in range(4)
                        ] + [
                            lambda nb=nb: _qT_blk(hp + 1, nb) for nb in range(2)
                        ]
                    pva = pvp.tile([DK + 1, 512], f32, tag="pv")
                    pvb = pvp.tile([DK + 1, 512], f32, tag="pv")
                    for g in range(NTK // 2):
                        if g < len(produce):
                            produce[g]()
                        sa = scp.tile([P, 2, 512], f32, tag="sc")
                        sb_ = scp.tile([P, 2, 512], f32, tag="sc")
                        for j in range(2):
                            tk = 2 * g + j
                            tksl = slice(tk * P, (tk + 1) * P)
                            nc.tensor.matmul(
                                sa[:, j, :],
                                lhsT=kT[0:DK, hp, tksl],
                                rhs=qT[0:DK, hp, qsl],
                                start=True,
                                stop=True,
                            )
                            nc.tensor.matmul(
                                sb_[:, j, :],
                                lhsT=kT[DK:P, hp, tksl],
                                rhs=qT[DK:P, hp, qsl],
                                start=True,
                                stop=True,
                            )
                        ea = expSp.tile([P, 2, 512], f8, tag="ea")
                        eb = expSp.tile([P, 2, 512], f8, tag="eb")
                        nc.scalar.activation(out=ea, in_=sa, func=AF.Exp, scale=SC_EXP)
                        nc.scalar.activation(out=eb, in_=sb_, func=AF.Exp, scale=SC_EXP)
                        nc.tensor.matmul(
                            pva,
                            lhsT=v_sb[:, 2 * hp, 2 * g : 2 * g + 2, 0 : DK + 1],
                            rhs=ea,
                            start=(g == 0),
                            stop=(g == NTK // 2 - 1),
                            skip_group_check=True,
                            perf_mode=mybir.MatmulPerfMode.DoubleRow,
                        )
                        nc.tensor.matmul(
                            pvb,
                            lhsT=v_sb[:, 2 * hp + 1, 2 * g : 2 * g + 2, 0 : DK + 1],
                            rhs=eb,
                            start=(g == 0),
                            stop=(g == NTK // 2 - 1),
                            skip_group_check=True,
                            perf_mode=mybir.MatmulPerfMode.DoubleRow,
                        )
                    # normalize by the summed 8.0-ones column (partition DK).
                    # Copy the unnormalized PV to SBUF so the PSUM tile is
                    # freed immediately; the reciprocal's partition-broadcast
                    # goes through DRAM on the SP queue, fully decoupled from
                    # the PE/exp pipeline (its only consumer is Wo at qb end).
                    fast = qb == 1 and hp == H // 2 - 1
                    for h_i, pv in ((0, pva), (1, pvb)):
                        rb = rbp.tile([1, 512], f32, tag="rb")
                        nc.vector.reciprocal(rb, pv[DK : DK + 1, :])
                        ou = rbp.tile([DK, 512], f32, tag="ou")
                        nc.vector.tensor_copy(out=ou, in_=pv[0:DK, :])
                        if fast:
                            # last head-pair gates the whole tail: skip the
                            # DRAM roundtrip, rank-1 broadcast on the now-idle
                            # PE into the slot the ou-copy just freed
                            rbB = pvp.tile([DK, 512], f32, tag="pv")
                            nc.tensor.matmul(
                                rbB, lhsT=ones64, rhs=rb, start=True, stop=True
                            )
                        else:
                            dr = dramp.tile([1, 512], f32, tag="dr")
                            nc.sync.dma_start(out=dr, in_=rb)
                            rbB = rbp.tile([DK, 512], f32, tag="rbB")
                            nc.sync.dma_start(
                                out=rbB, in_=dr[:, :].to_broadcast([DK, 512])
                            )
                        if h_i == 0:
                            nc.vector.tensor_mul(
                                out=oT[0:DK, hp, qsl], in0=ou, in1=rbB
                            )
                        else:
                            ot = rbp.tile([DK, 512], bf16, tag="ot")
                            nc.vector.tensor_mul(out=ot, in0=ou, in1=rbB)
                            nc.gpsimd.dma_start(out=oT[DK:P, hp, qsl], in_=ot)

            def _wo_qb(qb):
                # qb1's Wo is on the critical tail path: use the freed pv
                # ring (3 bufs) so the four Wo psums pipeline
                pool, tag = (pvp, "pv") if qb == 1 else (mmp, "ps")
                for tl in range(4):
                    tt = qb * 4 + tl
                    ps = pool.tile([P, 512], f32, tag=tag)
                    for ks in range(KS):
                        nc.tensor.matmul(
                            ps,
                            lhsT=oT[:, ks, tt * P : (tt + 1) * P],
                            rhs=wo_sb[:, ks, :],
                            start=(ks == 0),
                            stop=(ks == KS - 1),
                        )
                    nc.vector.tensor_add(
                        out=x_res[:, tt, :], in0=x_res[:, tt, :], in1=ps
                    )

            def _mlp1_half(half):
                hsl = slice(half * 512, (half + 1) * 512)
                for hc in range(HC):
                    hp_t = scp.tile([P, 512], f32, tag="sc")
                    for ks in range(0, KS, 2):
                        nc.tensor.matmul(
                            hp_t,
                            lhsT=w1_sb[:, ks : ks + 2, hc * P : (hc + 1) * P],
                            rhs=xn2T[:, ks : ks + 2, hsl],
                            start=(ks == 0),
                            stop=(ks == KS - 2),
                            perf_mode=mybir.MatmulPerfMode.DoubleRow,
                        )
                    gel_kw = {}
                    if has_bm1:
                        gel_kw["bias"] = bm1_sb[:, hc : hc + 1]
                    nc.scalar.activation(
                        out=hT[:, hc, hsl], in_=hp_t, func=AF.Gelu,
                        scale=1.0 / WS, **gel_kw,
                    )

            _attn_qb(0, produce_qkv=True)
            _wo_qb(0)
            for tl in range(4):
                _ln_tile(
                    x_res[:, tl, :], g2B, b2B, xn2T, tl, mmp,
                    act_copy=False, tp_tag="ps",
                )
            def _mlp2_tile(tt):
                o2 = pvp.tile([P, D], f32, tag="pv")
                for hc in range(0, HC, 2):
                    nc.tensor.matmul(
                        o2,
                        lhsT=hT[:, hc : hc + 2, tt * P : (tt + 1) * P],
                        rhs=w2_sb[:, hc : hc + 2, :],
                        start=(hc == 0),
                        stop=(hc == HC - 2),
                        perf_mode=mybir.MatmulPerfMode.DoubleRow,
                    )
                ot = outp.tile([P, D], f32, tag="out")
                nc.vector.scalar_tensor_tensor(
                    out=ot,
                    in0=o2,
                    scalar=1.0 / WS,
                    op0=OP.mult,
                    in1=x_res[:, tt, :],
                    op1=OP.add,
                )
                if has_bm2:
                    nc.vector.tensor_add(out=ot, in0=ot, in1=bm2B)
                nc.sync.dma_start(out=y_t[:, tt, :], in_=ot)

            _attn_qb(1, produce_qkv=False)
            # Tail order is chosen for the ACT queue: the qb1 LN2 rstds
            # (ln/exp set) run BEFORE the first gelu so ACT switches tables
            # exactly once; MLP1-h0's matmuls fill PE while qb1's Wo/LN2
            # chains drain on DVE.
            _wo_qb(1)
            xnbs = []
            for tl in range(4):
                tt = 4 + tl
                xnbs.append(_ln_vec(x_res[:, tt, :], g2B, b2B, tt))
            _mlp1_half(0)
            for tl in range(4):
                tt = 4 + tl
                _ln_tp(xnbs[tl], xn2T, tt, pvp, act_copy=False, tp_tag="pv")
            _mlp1_half(1)
            for tt in range(NTL):
                _mlp2_tile(tt)

    _split_excess_waits(nc)
    return nc


def _host_prep(inputs):
    x = np.asarray(inputs["x"], np.float32)
    c = np.asarray(inputs["c"], np.float32)
    mod = (c / (1.0 + np.exp(-c))) @ np.asarray(inputs["W_ada"], np.float32)
    mod = mod + np.asarray(inputs["b_ada"], np.float32)
    scale_msa, gate_msa, scale_mlp, gate_mlp = np.split(mod, 4, axis=-1)

    g1 = np.asarray(inputs["g1"], np.float32)
    b1 = np.asarray(inputs["b1"], np.float32)
    g2 = np.asarray(inputs["g2"], np.float32)
    b2 = np.asarray(inputs["b2"], np.float32)
    Wo = np.asarray(inputs["Wo"], np.float32)
    W2 = np.asarray(inputs["W_mlp2"], np.float32)
    bm2 = np.asarray(inputs["b_mlp2"], np.float32)

    wq8 = (np.asarray(inputs["Wq"], np.float32) * WS).astype(F8E4)
    wk8 = (np.asarray(inputs["Wk"], np.float32) * WS).astype(F8E4)
    wv8 = (np.asarray(inputs["Wv"], np.float32) * WS).astype(F8E4)
    w18 = (np.asarray(inputs["W_mlp1"], np.float32) * WS).astype(F8E4)

    per_core = []
    for core in range(NCORES):
        b = core // 2
        half = core % 2
        s1 = 1.0 + scale_msa[b]
        s2 = 1.0 + scale_mlp[b]
        m = {
            "xm": np.ascontiguousarray(x[b, half * TL : (half + 1) * TL]),
            "xo": np.ascontiguousarray(x[b, (1 - half) * TL : (2 - half) * TL]).astype(BF16),
            "wq": wq8,
            "wk": wk8,
            "wv": wv8,
            "wo": (Wo * gate_msa[b][None, :]).astype(BF16),
            "w1": w18,
            "w2": (W2 * (WS * gate_mlp[b][None, :])).astype(F8E4),
            "g1": (g1 * s1).astype(np.float32),
            "g2": (g2 * s2).astype(np.float32),
        }
        _b1 = (b1 * s1).astype(np.float32)
        _b2 = (b2 * s2).astype(np.float32)
        _bm1 = np.asarray(inputs["b_mlp1"], np.float32)
        _bm2 = (bm2 * gate_mlp[b]).astype(np.float32)
        m["_flags"] = (
            bool(np.any(_b1)), bool(np.any(_b2)),
            bool(np.any(_bm1)), bool(np.any(_bm2)),
        )
        if m["_flags"][0]:
            m["b1"] = _b1
        if m["_flags"][1]:
            m["b2"] = _b2
        if m["_flags"][2]:
            m["bm1"] = _bm1
        if m["_flags"][3]:
            m["bm2"] = _bm2
        per_core.append(m)
    return per_core


def kernel(**inputs):
    from concourse import bass_utils

    per_core = _host_prep(inputs)
    flags = per_core[0]["_flags"]
    for m in per_core:
        assert m["_flags"] == flags
        del m["_flags"]

    if ("nc", flags) not in _cache:
        _cache[("nc", flags)] = _build_program(*flags)
    nc = _cache[("nc", flags)]

    res = bass_utils.run_bass_kernel_spmd(nc, per_core, core_ids=list(range(NCORES)))

    x = inputs["x"]
    out = np.empty((x.shape[0], T, D), np.float32)
    for core in range(NCORES):
        b = core // 2
        half = core % 2
        out[b, half * TL : (half + 1) * TL] = res.results[core]["y"]
    return out

